# revision 7
# baseline (speedup 1.0000x reference)
"""FJSP decoder kernel for Trainium2, data-parallel over batch on 8 NeuronCores.

Key algebraic restructuring: q/k/v for the flattened (job, machine) pair
s=(j,m) decompose as x[s] = xj[j] + xm[m], so the joint-axis attention
softmax factorizes exactly:

  score[s, (j',m')] = E[s,j'] + F[s,m']      (E from A,C; F from B,Dm)
  softmax_t(score) @ v = softmax_j'(E) @ vj + softmax_m'(F) @ vm

and with E[(j,m),j'] = (A[j,j'] + C[m,j'])/sqrt(QD) the row softmax of E
itself factorizes through exp(A)*exp(C), giving per head only J*J-sized
matmuls -- the [S,S] = [2000,2000] score matrix is never materialized.
The multi-head combine collapses through w2 = Wmhc @ Wshc into per-head
scalars uv = v @ w2, so the whole decoder reduces to [100,20]-shaped work:

  SE|Nj = eAT.T @ [eCT | eCT*uvj];  SF|Nm = eBT.T @ [eDT | eDT*uvm]
  score1 = (sum_h Nj/SE + Nm/SF + bias)/sqrt(D)
  p = softmax_flat(10*tanh(score1) + mask)   (tanh via exp, one ACT table)

Layout notes: heads are padded to 32-partition strips (two groups of 4
heads) so per-head K=16 contractions become legal K=32 matmuls at base
partitions {0,32,64,96}; k/v projection tiles carry 80 zero columns so
every per-head matmul runs in the same (32-row, 128-col) PE tiling mode.
"""

import math

import numpy as np

import concourse.bass as bass
import concourse.mybir as mybir
import concourse.tile as tile
from concourse.bass_utils import run_bass_kernel_spmd
from concourse.masks import make_identity

F32 = mybir.dt.float32
AF = mybir.ActivationFunctionType
OP = mybir.AluOpType
AX = mybir.AxisListType

D, H, QD = 128, 8, 16
B, J, M = 8, 100, 20
HQ = H * QD  # 128
INV_SQ = 1.0 / math.sqrt(QD)  # 0.25
SD = math.sqrt(D)

# ---------------------------------------------------------------------------
# gen3 walrus accepts one sync-wait per instruction. Tile's kernel-tail
# drain accumulates one wait per active logical processor on a single
# Drain: spread them across engines (parallel waiting). Tile's semaphore
# pass can also attach >1 wait to ordinary instructions: shed extras onto
# same-engine NoOps inserted right before the offender.
_PATCHED = False


def _install_drain_patch():
    global _PATCHED
    if _PATCHED:
        return
    from concourse.tile import ScopedClock, TileContext

    def _split_drain_and_barrier(self, tick_clock, wait_clock):
        drain_inst = self.nc.sync.drain()
        wait_clock.add_sem_waits(
            drain_inst.ins, ScopedClock({None: tick_clock.global_clock})
        )
        si = drain_inst.ins.sync_info
        waits = list(si.on_wait) if si is not None else []
        if len(waits) > 1:
            assert not si.on_update
            sems = {s.name: s for s in self.sems.allocated().values()}
            drain_inst.ins.sync_info = None
            drain_inst.wait_op(sems[waits[0].ant_name], waits[0].wait_value, "sem-ge")
            engines = [
                self.nc.scalar,
                self.nc.vector,
                self.nc.tensor,
                self.nc.gpsimd,
                self.nc.sync,
            ]
            for i, w in enumerate(waits[1:]):
                extra = engines[i % len(engines)].drain()
                extra.wait_op(sems[w.ant_name], w.wait_value, "sem-ge")
        self.nc.all_engine_barrier()
        assert self.sems is not None
        popped = self.nc._tile_sem_poison_stack.pop()
        assert popped is self._sem_poison
        self.nc.clear_and_free_semaphores(list(self.sems.allocated().values()))

    TileContext._drain_and_barrier = _split_drain_and_barrier
    _PATCHED = True


def _split_multi_waits(nc):
    import bass_rust

    ctr = 0
    for fn in nc.m.functions:
        for bb in fn.blocks:
            il = bb.instructions
            if not any(
                i.sync_info is not None and len(i.sync_info.on_wait) > 1 for i in il
            ):
                continue
            new = []
            for ins in il:
                si = ins.sync_info
                if si is not None and len(si.on_wait) > 1:
                    waits = list(si.on_wait)
                    ups = list(si.on_update)
                    for w in waits[:-1]:
                        nop = mybir.InstNoOp(name=f"I-waitsplit-{ctr}", ins=[], outs=[])
                        ctr += 1
                        nop.engine = ins.engine
                        nop.sync_info = bass_rust.SyncInfo(on_update=[], on_wait=[w])
                        new.append(nop)
                    ins.sync_info = bass_rust.SyncInfo(
                        on_update=ups, on_wait=[waits[-1]]
                    )
                new.append(ins)
            bb.instructions = new


def _chunk2(ap_slice, chunk_step):
    """Matmul rhs built from two `w`-wide column chunks `chunk_step` apart."""
    return bass.AP(
        tensor=ap_slice.tensor,
        offset=ap_slice.offset,
        ap=[ap_slice.ap[0], [chunk_step, 2], ap_slice.ap[1]],
    )


def _build():
    nc = bass.Bass()
    ej_d = nc.dram_tensor("ej", [J, D], F32, kind="ExternalInput")
    em_d = nc.dram_tensor("em", [M, D], F32, kind="ExternalInput")
    mask_d = nc.dram_tensor("mask", [J, M], F32, kind="ExternalInput")
    wq_d = nc.dram_tensor("Wq3", [2 * D, HQ], F32, kind="ExternalInput")
    wk_d = nc.dram_tensor("Wk", [2 * D, HQ], F32, kind="ExternalInput")
    wv_d = nc.dram_tensor("Wv", [2 * D, HQ], F32, kind="ExternalInput")
    wmhc_d = nc.dram_tensor("Wmhc", [HQ, D], F32, kind="ExternalInput")
    # host-packed: col0 = b_mhc, col1 = Wshc, col2[0] = b_shc
    smw_d = nc.dram_tensor("smallw", [D, 3], F32, kind="ExternalInput")
    out_d = nc.dram_tensor("out", [J, M], F32, kind="ExternalOutput")

    with tile.TileContext(nc) as tc:
        with (
            tc.tile_pool(name="persist", bufs=1) as pp,
            tc.tile_pool(name="rot", bufs=6) as rp,
            tc.tile_pool(name="ps_big", bufs=2, space="PSUM") as ps_big,
            tc.tile_pool(name="ps_s1", bufs=4, space="PSUM") as ps_s1,
            tc.tile_pool(name="ps_small", bufs=2, space="PSUM") as ps_small,
        ):
            # ---- constants that gate the PE transposes ------------------
            ident = pp.tile([D, D], F32, tag="ident")
            make_identity(nc, ident)

            # ---- input DMAs spread across engine queues, critical first --
            ej_sb = pp.tile([J, D], F32, tag="ej")
            nc.scalar.dma_start(out=ej_sb, in_=ej_d[:])
            w_sb = {}
            for nm, dt_, eng in (
                ("q", wq_d, nc.sync),
                ("k", wk_d, nc.scalar),
                ("v", wv_d, nc.sync),
            ):
                t = pp.tile([D, 2, HQ], F32, tag=f"w{nm}")
                eng.dma_start(out=t, in_=dt_[:].rearrange("(two d) o -> d two o", two=2))
                w_sb[nm] = t
            em_sb = pp.tile([M, D], F32, tag="em")
            nc.scalar.dma_start(out=em_sb, in_=em_d[:])
            wmhc_sb = pp.tile([HQ, D], F32, tag="wmhc")
            nc.sync.dma_start(out=wmhc_sb, in_=wmhc_d[:])
            smw_sb = pp.tile([D, 3], F32, tag="smw")
            nc.sync.dma_start(out=smw_sb, in_=smw_d[:])
            mask_sb = pp.tile([J, M], F32, tag="mask")
            nc.sync.dma_start(out=mask_sb, in_=mask_d[:])

            # k/v projection tiles get 80 zero cols (120:200) so machine-
            # side per-head matmuls run with M=100 (128-col PE mode)
            pT_sb = {}
            for nm in ("q", "k", "v"):
                for grp in range(2):
                    w = 120 if nm == "q" else 200
                    sb = pp.tile([D, w], F32, tag=f"{nm}T{grp}")
                    if nm != "q":
                        nc.gpsimd.memset(sb[:, 120:200], 0.0)
                    pT_sb[(nm, grp)] = sb

            ones_sb = pp.tile([D, D], F32, tag="ones")
            nc.gpsimd.memset(ones_sb, 1.0)

            # padded weights: head h -> 32-strip 32g..32g+16 (g = h%4) in
            # group A (h<4) / B (h>=4); the other 16 lanes zero.
            wpad = pp.tile([D, 12, D], F32, tag="wpad")
            wpad_idx = {}
            idx = 0
            for nm in ("q", "k", "v"):
                for half in range(2):
                    for grp in range(2):
                        wpad_idx[(nm, half, grp)] = idx
                        eng = nc.vector if nm == "q" else nc.gpsimd
                        tv = wpad[:, idx, :].rearrange("p (g c) -> p g c", c=32)
                        eng.memset(tv[:, :, 16:32], 0.0)
                        src = w_sb[nm][:, half, grp * 64 : (grp + 1) * 64].rearrange(
                            "p (g c) -> p g c", c=16
                        )
                        eng.tensor_copy(out=tv[:, :, 0:16], in_=src)
                        idx += 1

            # ---- PE transposes ------------------------------------------
            ejT_ps = ps_big.tile([D, 320], F32, tag="big")
            nc.tensor.transpose(ejT_ps[:, 0:J], ej_sb, ident[0:J, 0:J])
            ejT_sb = pp.tile([D, J], F32, tag="ejT")
            nc.scalar.copy(out=ejT_sb, in_=ejT_ps[:, 0:J])

            emT_ps = ps_big.tile([D, 320], F32, tag="big")
            nc.tensor.transpose(emT_ps[:, 0:M], em_sb, ident[0:M, 0:M])
            emT_sb = pp.tile([D, M], F32, tag="emT")
            nc.scalar.copy(out=emT_sb, in_=emT_ps[:, 0:M])

            wmhcT_ps = ps_big.tile([D, 320], F32, tag="big")
            nc.tensor.transpose(wmhcT_ps[:, 0:HQ], wmhc_sb, ident)
            wmhcT_sb = pp.tile([D, HQ], F32, tag="wmhcT")
            nc.scalar.copy(out=wmhcT_sb, in_=wmhcT_ps[:, 0:HQ])

            # WmhcT with columns in padded-head layout, per group
            wmhcPT = pp.tile([D, 2, D], F32, tag="wmhcPT")
            for grp in range(2):
                tv = wmhcPT[:, grp, :].rearrange("p (g c) -> p g c", c=32)
                nc.gpsimd.memset(tv[:, :, 16:32], 0.0)
                src = wmhcT_sb[:, grp * 64 : (grp + 1) * 64].rearrange(
                    "p (g c) -> p g c", c=16
                )
                nc.gpsimd.tensor_copy(out=tv[:, :, 0:16], in_=src)

            # ---- 128x128 mode: projections, w2pad, bias -----------------
            for nm in ("q", "k", "v"):
                for grp in range(2):
                    ps = ps_big.tile([D, 320], F32, tag="big")
                    nc.tensor.matmul(
                        out=ps[:, 0:J],
                        lhsT=wpad[:, wpad_idx[(nm, 0, grp)], :],
                        rhs=ejT_sb,
                    )
                    nc.tensor.matmul(
                        out=ps[:, J : J + M],
                        lhsT=wpad[:, wpad_idx[(nm, 1, grp)], :],
                        rhs=emT_sb,
                    )
                    sb = pT_sb[(nm, grp)]
                    eng = nc.scalar if nm in ("q", "k") else nc.vector
                    if nm == "q":
                        eng.copy(out=sb[:, 0:120], in_=ps[:, 0:120])
                    elif nm == "k":
                        eng.copy(out=sb[:, 0:120], in_=ps[:, 0:120])
                    else:
                        eng.tensor_copy(out=sb[:, 0:120], in_=ps[:, 0:120])

            w2pad_sb = []
            for grp in range(2):
                ps = ps_small.tile([D, 8], F32, tag="small")
                nc.tensor.matmul(
                    out=ps[:, 0:1], lhsT=wmhcPT[:, grp, :], rhs=smw_sb[:, 1:2]
                )
                sb = pp.tile([D, 1], F32, tag=f"w2pad{grp}")
                nc.vector.tensor_copy(out=sb, in_=ps[:, 0:1])
                w2pad_sb.append(sb)

            # bias_const = b_mhc @ Wshc + b_shc, broadcast over J partitions
            bw = pp.tile([D, 1], F32, tag="bw")
            nc.vector.tensor_mul(out=bw, in0=smw_sb[:, 0:1], in1=smw_sb[:, 1:2])
            nc.vector.tensor_add(out=bw[0:1, 0:1], in0=bw[0:1, 0:1], in1=smw_sb[0:1, 2:3])
            bias_ps = ps_small.tile([D, 8], F32, tag="small")
            nc.tensor.matmul(out=bias_ps[0:J, 0:1], lhsT=ones_sb[:, 0:J], rhs=bw)
            biasb = pp.tile([J, 1], F32, tag="biasb")
            nc.scalar.mul(out=biasb, in_=bias_ps[0:J, 0:1], mul=2.0 / SD)

            # ---- (32,128) mode: uv vectors + per-head products ----------
            uvj_ps = ps_small.tile([D, 8], F32, tag="small")
            uvm_ps = ps_small.tile([D, 8], F32, tag="small")
            for h in range(H):
                grp, g = divmod(h, 4)
                vt = pT_sb[("v", grp)]
                nc.tensor.matmul(
                    out=uvj_ps[0:J, h : h + 1],
                    lhsT=vt[32 * g : 32 * g + 32, 0:J],
                    rhs=w2pad_sb[grp][32 * g : 32 * g + 32, :],
                    tile_position=(32 * g, 0),
                )
                nc.tensor.matmul(
                    out=uvm_ps[0:J, h : h + 1],
                    lhsT=vt[32 * g : 32 * g + 32, 100:200],
                    rhs=w2pad_sb[grp][32 * g : 32 * g + 32, :],
                    tile_position=(32 * g, 0),
                )
            uvj_sb = pp.tile([J, H], F32, tag="uvj")
            nc.vector.tensor_copy(out=uvj_sb, in_=uvj_ps[0:J, 0:H])
            uvm_sb = pp.tile([M, H], F32, tag="uvm")
            nc.vector.tensor_copy(out=uvm_sb, in_=uvm_ps[0:M, 0:H])

            # per head: o_ps = [AT|CT | BT|DT(+zeros)], one exp, uv scales
            eE = []
            for h in range(H):
                grp, g = divmod(h, 4)
                kt, qt = pT_sb[("k", grp)], pT_sb[("q", grp)]
                ps = ps_s1.tile([D, 240], F32, tag="s1")
                nc.tensor.matmul(
                    out=ps[0:J, 0:120],
                    lhsT=kt[32 * g : 32 * g + 32, 0:J],
                    rhs=qt[32 * g : 32 * g + 32, 0:120],
                    tile_position=(32 * g, 0),
                )
                nc.tensor.matmul(
                    out=ps[0:J, 120:240],
                    lhsT=kt[32 * g : 32 * g + 32, 100:200],
                    rhs=qt[32 * g : 32 * g + 32, 0:120],
                    tile_position=(32 * g, 0),
                )
                e1 = rp.tile([D, 280], F32, tag="eE")
                nc.scalar.activation(
                    out=e1[0:J, 0:240], in_=ps[0:J, 0:240], func=AF.Exp, scale=INV_SQ
                )
                nc.vector.tensor_scalar_mul(
                    out=e1[0:J, 240:260],
                    in0=e1[0:J, 100:120],
                    scalar1=uvj_sb[:, h : h + 1],
                )
                nc.vector.tensor_scalar_mul(
                    out=e1[0:M, 260:280],
                    in0=e1[0:M, 220:240],
                    scalar1=uvm_sb[:, h : h + 1],
                )
                eE.append(e1)

            # mm4: [SF|Nm] = eBT.T @ [eDT | eDT*uvm]   (K=20, (32,128))
            f_ps = ps_big.tile([D, 8, 40], F32, tag="big")
            for h in range(H):
                nc.tensor.matmul(
                    out=f_ps[0:J, h, :],
                    lhsT=eE[h][0:M, 120:220],
                    rhs=_chunk2(eE[h][0:M, 220:240], 40),
                )

            def pmh(ap3):  # [p, h, m] -> [p, m, h]
                return ap3.rearrange("p h m -> p m h")

            # F-side combine first: overlaps the mm3 stream on PE
            rF = pp.tile([J, M, H], F32, tag="rF")
            nc.vector.reciprocal(out=rF, in_=pmh(f_ps[0:J, :, 0:M]))
            d2 = pp.tile([J, M, H], F32, tag="d2")
            nc.vector.tensor_mul(out=d2, in0=pmh(f_ps[0:J, :, M : 2 * M]), in1=rF)

            # mm3: [SE|Nj] = eAT.T @ [eCT | eCT*uvj]   (K=100, (128,128))
            s_ps = ps_big.tile([D, 8, 40], F32, tag="big")
            for h in range(H):
                nc.tensor.matmul(
                    out=s_ps[0:J, h, :],
                    lhsT=eE[h][0:J, 0:J],
                    rhs=_chunk2(eE[h][0:J, 100:120], 140),
                )

            # ---- combine: sum_h Nj/SE + Nm/SF ---------------------------
            rE = pp.tile([J, M, H], F32, tag="rE")
            nc.vector.reciprocal(out=rE, in_=pmh(s_ps[0:J, :, 0:M]))
            c8 = pp.tile([J, M, H], F32, tag="c8")
            nc.vector.scalar_tensor_tensor(
                out=c8, in0=pmh(s_ps[0:J, :, M : 2 * M]), scalar=1.0, in1=rE,
                op0=OP.mult, op1=OP.mult,
            )
            nc.vector.tensor_add(out=c8, in0=c8, in1=d2)
            c1 = pp.tile([J, M], F32, tag="c1")
            nc.vector.reduce_sum(out=c1, in_=c8, axis=AX.X)

            # tanh chain via exp (no ACT table switch):
            # logits ~ mask - 20/(exp(2*(c1+bias)/sqrt(D)) + 1)  (+const)
            u = pp.tile([J, M], F32, tag="u")
            nc.scalar.activation(out=u, in_=c1, func=AF.Exp, scale=2.0 / SD, bias=biasb)
            t1 = pp.tile([J, M], F32, tag="t1")
            nc.scalar.add(out=t1, in_=u, add=1.0)
            r = pp.tile([J, M], F32, tag="r")
            nc.vector.reciprocal(out=r, in_=t1)
            arg = pp.tile([J, M], F32, tag="arg")
            nc.vector.scalar_tensor_tensor(
                out=arg, in0=r, scalar=-20.0, in1=mask_sb, op0=OP.mult, op1=OP.add
            )
            e_sb = pp.tile([J, M], F32, tag="e")
            s_row = pp.tile([J, 1], F32, tag="srow")
            nc.scalar.activation(
                out=e_sb, in_=arg, func=AF.Exp, scale=1.0, accum_out=s_row
            )
            totb_ps = ps_small.tile([D, 8], F32, tag="small")
            nc.tensor.matmul(out=totb_ps[0:J, 0:1], lhsT=ones_sb[0:J, 0:J], rhs=s_row)
            rtot = pp.tile([J, 1], F32, tag="rtot")
            nc.vector.reciprocal(out=rtot, in_=totb_ps[0:J, 0:1])
            out_t = pp.tile([J, M], F32, tag="outt")
            nc.vector.tensor_scalar_mul(out=out_t, in0=e_sb, scalar1=rtot)
            nc.sync.dma_start(out=out_d[:], in_=out_t)

    _split_multi_waits(nc)
    return nc


_NC = None
last_results = None


def kernel(**inputs):
    global _NC, last_results
    _install_drain_patch()
    if _NC is None:
        _NC = _build()

    f32 = lambda a: np.ascontiguousarray(np.asarray(a), dtype=np.float32)
    smallw = np.zeros((D, 3), np.float32)
    smallw[:, 0] = np.asarray(inputs["b_mhc"], np.float32).reshape(D)
    smallw[:, 1] = np.asarray(inputs["Wshc"], np.float32).reshape(D)
    smallw[0, 2] = np.float32(np.asarray(inputs["b_shc"]).reshape(-1)[0])
    shared = {
        "Wq3": f32(inputs["Wq3"]),
        "Wk": f32(inputs["Wk"]),
        "Wv": f32(inputs["Wv"]),
        "Wmhc": f32(inputs["Wmhc"]),
        "smallw": smallw,
    }
    in_maps = []
    for b in range(B):
        m = dict(shared)
        m["ej"] = f32(inputs["encoded_job"][b])
        m["em"] = f32(inputs["encoded_machine"][b])
        m["mask"] = f32(inputs["ninf_mask"][b])
        in_maps.append(m)

    last_results = run_bass_kernel_spmd(_NC, in_maps, core_ids=list(range(B)))
    out = np.stack(
        [last_results.results[b]["out"].reshape(J * M) for b in range(B)]
    )
    return out.astype(np.float32)


# revision 8
# speedup vs baseline: 1.0491x; 1.0491x over previous
"""FJSP decoder kernel for Trainium2, data-parallel over batch on 8 NeuronCores.

Key algebraic restructuring: q/k/v for the flattened (job, machine) pair
s=(j,m) decompose as x[s] = xj[j] + xm[m], so the joint-axis attention
softmax factorizes exactly:

  score[s, (j',m')] = E[s,j'] + F[s,m']      (E from A,C; F from B,Dm)
  softmax_t(score) @ v = softmax_j'(E) @ vj + softmax_m'(F) @ vm

and with E[(j,m),j'] = (A[j,j'] + C[m,j'])/sqrt(QD) the row softmax of E
itself factorizes through exp(A)*exp(C), giving per head only J*J-sized
matmuls -- the [S,S] = [2000,2000] score matrix is never materialized.
The multi-head combine collapses through w2 = Wmhc @ Wshc into per-head
scalars uv = v @ w2, so the whole decoder reduces to [100,20]-shaped work:

  SE|Nj = eAT.T @ [eCT | eCT*uvj];  SF|Nm = eBT.T @ [eDT | eDT*uvm]
  score1 = (sum_h Nj/SE + Nm/SF + bias)/sqrt(D)
  p = softmax_flat(10*tanh(score1) + mask)   (tanh via exp, one ACT table)

Layout notes: heads are padded to 32-partition strips (two groups of 4
heads) so per-head K=16 contractions become legal K=32 matmuls at base
partitions {0,32,64,96}; k/v projection tiles carry 80 zero columns so
every per-head matmul runs in the same (32-row, 128-col) PE tiling mode.
All inputs are host-packed into two DRAM tensors (weights, activations)
so the kernel issues exactly two input DMAs.
"""

import math

import numpy as np

import concourse.bass as bass
import concourse.mybir as mybir
import concourse.tile as tile
from concourse.bass_utils import run_bass_kernel_spmd
from concourse.masks import make_identity

F32 = mybir.dt.float32
AF = mybir.ActivationFunctionType
OP = mybir.AluOpType
AX = mybir.AxisListType

D, H, QD = 128, 8, 16
B, J, M = 8, 100, 20
HQ = H * QD  # 128
INV_SQ = 1.0 / math.sqrt(QD)  # 0.25
SD = math.sqrt(D)

# edata column layout: [ej 0:128 | em 128:256 | mask 256:276 | smallw 276:279]
EJ0, EM0, MK0, SW0 = 0, 128, 256, 276
EDATA_W = 279

# ---------------------------------------------------------------------------
# gen3 walrus accepts one sync-wait per instruction. Tile's kernel-tail
# drain accumulates one wait per active logical processor on a single
# Drain: spread them across engines (parallel waiting). Tile's semaphore
# pass can also attach >1 wait to ordinary instructions: shed extras onto
# same-engine NoOps inserted right before the offender.
_PATCHED = False


def _install_drain_patch():
    global _PATCHED
    if _PATCHED:
        return
    from concourse.tile import ScopedClock, TileContext

    def _split_drain_and_barrier(self, tick_clock, wait_clock):
        drain_inst = self.nc.sync.drain()
        wait_clock.add_sem_waits(
            drain_inst.ins, ScopedClock({None: tick_clock.global_clock})
        )
        si = drain_inst.ins.sync_info
        waits = list(si.on_wait) if si is not None else []
        if len(waits) > 1:
            assert not si.on_update
            sems = {s.name: s for s in self.sems.allocated().values()}
            drain_inst.ins.sync_info = None
            drain_inst.wait_op(sems[waits[0].ant_name], waits[0].wait_value, "sem-ge")
            engines = [
                self.nc.scalar,
                self.nc.vector,
                self.nc.tensor,
                self.nc.gpsimd,
                self.nc.sync,
            ]
            for i, w in enumerate(waits[1:]):
                extra = engines[i % len(engines)].drain()
                extra.wait_op(sems[w.ant_name], w.wait_value, "sem-ge")
        self.nc.all_engine_barrier()
        assert self.sems is not None
        popped = self.nc._tile_sem_poison_stack.pop()
        assert popped is self._sem_poison
        self.nc.clear_and_free_semaphores(list(self.sems.allocated().values()))

    TileContext._drain_and_barrier = _split_drain_and_barrier
    _PATCHED = True


def _split_multi_waits(nc):
    import bass_rust

    ctr = 0
    for fn in nc.m.functions:
        for bb in fn.blocks:
            il = bb.instructions
            if not any(
                i.sync_info is not None and len(i.sync_info.on_wait) > 1 for i in il
            ):
                continue
            new = []
            for ins in il:
                si = ins.sync_info
                if si is not None and len(si.on_wait) > 1:
                    waits = list(si.on_wait)
                    ups = list(si.on_update)
                    for w in waits[:-1]:
                        nop = mybir.InstNoOp(name=f"I-waitsplit-{ctr}", ins=[], outs=[])
                        ctr += 1
                        nop.engine = ins.engine
                        nop.sync_info = bass_rust.SyncInfo(on_update=[], on_wait=[w])
                        new.append(nop)
                    ins.sync_info = bass_rust.SyncInfo(
                        on_update=ups, on_wait=[waits[-1]]
                    )
                new.append(ins)
            bb.instructions = new


def _chunk2(ap_slice, chunk_step):
    """Matmul rhs built from two equal column chunks `chunk_step` apart."""
    return bass.AP(
        tensor=ap_slice.tensor,
        offset=ap_slice.offset,
        ap=[ap_slice.ap[0], [chunk_step, 2], ap_slice.ap[1]],
    )


def _build():
    nc = bass.Bass()
    # wqkv[:, i, :]: 0=Wq3-job 1=Wq3-mach 2=Wk-job 3=Wk-mach 4=Wv-job
    # 5=Wv-mach 6=Wmhc
    wqkv_d = nc.dram_tensor("wqkv", [D, 7, D], F32, kind="ExternalInput")
    ed_d = nc.dram_tensor("edata", [D, EDATA_W], F32, kind="ExternalInput")
    out_d = nc.dram_tensor("out", [J, M], F32, kind="ExternalOutput")

    with tile.TileContext(nc) as tc:
        with (
            tc.tile_pool(name="persist", bufs=1) as pp,
            tc.tile_pool(name="rot", bufs=6) as rp,
            tc.tile_pool(name="ps_big", bufs=2, space="PSUM") as ps_big,
            tc.tile_pool(name="ps_s1", bufs=6, space="PSUM") as ps_s1,
        ):
            # ---- constants that gate the PE transposes ------------------
            ident = pp.tile([D, D], F32, tag="ident")
            make_identity(nc, ident)

            # ---- the two input DMAs -------------------------------------
            ed_sb = pp.tile([D, EDATA_W], F32, tag="edata")
            nc.scalar.dma_start(out=ed_sb, in_=ed_d[:])
            wqkv_sb = pp.tile([D, 7, D], F32, tag="wqkv")
            nc.sync.dma_start(out=wqkv_sb, in_=wqkv_d[:])

            ej_v = ed_sb[0:J, EJ0 : EJ0 + D]
            em_v = ed_sb[0:M, EM0 : EM0 + D]
            mask_v = ed_sb[0:J, MK0 : MK0 + M]
            bmhc_v = ed_sb[:, SW0 : SW0 + 1]
            wshc_v = ed_sb[:, SW0 + 1 : SW0 + 2]
            bshc_v = ed_sb[0:1, SW0 + 2 : SW0 + 3]

            # k/v projection tiles get 80 zero cols (120:200) so machine-
            # side per-head matmuls run with M=100 (128-col PE mode)
            pT_sb = {}
            for nm in ("q", "k", "v"):
                for grp in range(2):
                    w = 120 if nm == "q" else 200
                    sb = pp.tile([D, w], F32, tag=f"{nm}T{grp}")
                    if nm != "q":
                        nc.gpsimd.memset(sb[:, 120:200], 0.0)
                    pT_sb[(nm, grp)] = sb

            ones_sb = pp.tile([D, D], F32, tag="ones")
            nc.gpsimd.memset(ones_sb, 1.0)

            # padded weights: head h -> 32-strip 32g..32g+16 (g = h%4) in
            # group A (h<4) / B (h>=4); the other 16 lanes zero.
            wpad = pp.tile([D, 12, D], F32, tag="wpad")
            wpad_idx = {}
            idx = 0
            for nm_i, nm in enumerate(("q", "k", "v")):
                for half in range(2):
                    for grp in range(2):
                        wpad_idx[(nm, half, grp)] = idx
                        eng = nc.vector if nm == "q" else nc.gpsimd
                        tv = wpad[:, idx, :].rearrange("p (g c) -> p g c", c=32)
                        eng.memset(tv[:, :, 16:32], 0.0)
                        src = wqkv_sb[
                            :, nm_i * 2 + half, grp * 64 : (grp + 1) * 64
                        ].rearrange("p (g c) -> p g c", c=16)
                        eng.tensor_copy(out=tv[:, :, 0:16], in_=src)
                        idx += 1

            # ---- PE transposes ------------------------------------------
            ejT_ps = ps_big.tile([D, 320], F32, tag="big")
            nc.tensor.transpose(ejT_ps[:, 0:J], ej_v, ident[0:J, 0:J])
            ejT_sb = pp.tile([D, J], F32, tag="ejT")
            nc.scalar.copy(out=ejT_sb, in_=ejT_ps[:, 0:J])

            emT_ps = ps_big.tile([D, 320], F32, tag="big")
            nc.tensor.transpose(emT_ps[:, 0:M], em_v, ident[0:M, 0:M])
            emT_sb = pp.tile([D, M], F32, tag="emT")
            nc.scalar.copy(out=emT_sb, in_=emT_ps[:, 0:M])

            wmhcT_ps = ps_big.tile([D, 320], F32, tag="big")
            nc.tensor.transpose(wmhcT_ps[:, 0:HQ], wqkv_sb[:, 6, :], ident)
            wmhcT_sb = pp.tile([D, HQ], F32, tag="wmhcT")
            nc.scalar.copy(out=wmhcT_sb, in_=wmhcT_ps[:, 0:HQ])

            # WmhcT with columns in padded-head layout, per group
            wmhcPT = pp.tile([D, 2, D], F32, tag="wmhcPT")
            for grp in range(2):
                tv = wmhcPT[:, grp, :].rearrange("p (g c) -> p g c", c=32)
                nc.gpsimd.memset(tv[:, :, 16:32], 0.0)
                src = wmhcT_sb[:, grp * 64 : (grp + 1) * 64].rearrange(
                    "p (g c) -> p g c", c=16
                )
                nc.gpsimd.tensor_copy(out=tv[:, :, 0:16], in_=src)

            # ---- 128x128 mode: projections, w2pad, bias -----------------
            for nm in ("q", "k", "v"):
                for grp in range(2):
                    ps = ps_big.tile([D, 320], F32, tag="big")
                    nc.tensor.matmul(
                        out=ps[:, 0:J],
                        lhsT=wpad[:, wpad_idx[(nm, 0, grp)], :],
                        rhs=ejT_sb,
                    )
                    nc.tensor.matmul(
                        out=ps[:, J : J + M],
                        lhsT=wpad[:, wpad_idx[(nm, 1, grp)], :],
                        rhs=emT_sb,
                    )
                    sb = pT_sb[(nm, grp)]
                    if nm == "v":
                        nc.vector.tensor_copy(out=sb[:, 0:120], in_=ps[:, 0:120])
                    else:
                        nc.scalar.copy(out=sb[:, 0:120], in_=ps[:, 0:120])

            w2pad_sb = []
            for grp in range(2):
                ps = ps_big.tile([D, 320], F32, tag="big")
                nc.tensor.matmul(
                    out=ps[:, 0:1], lhsT=wmhcPT[:, grp, :], rhs=wshc_v
                )
                sb = pp.tile([D, 1], F32, tag=f"w2pad{grp}")
                nc.vector.tensor_copy(out=sb, in_=ps[:, 0:1])
                w2pad_sb.append(sb)

            # bias_const = b_mhc @ Wshc + b_shc, broadcast over J partitions
            bw = pp.tile([D, 1], F32, tag="bw")
            nc.vector.tensor_mul(out=bw, in0=bmhc_v, in1=wshc_v)
            nc.vector.tensor_add(out=bw[0:1, 0:1], in0=bw[0:1, 0:1], in1=bshc_v)
            bias_ps = ps_big.tile([D, 320], F32, tag="big")
            nc.tensor.matmul(out=bias_ps[0:J, 0:1], lhsT=ones_sb[:, 0:J], rhs=bw)
            biasb = pp.tile([J, 1], F32, tag="biasb")
            nc.scalar.mul(out=biasb, in_=bias_ps[0:J, 0:1], mul=2.0 / SD)

            # ---- (32,128) mode: uv vectors + per-head products ----------
            uvj_ps = ps_big.tile([D, 320], F32, tag="big")
            uvm_ps = ps_big.tile([D, 320], F32, tag="big")
            for h in range(H):
                grp, g = divmod(h, 4)
                vt = pT_sb[("v", grp)]
                nc.tensor.matmul(
                    out=uvj_ps[0:J, h : h + 1],
                    lhsT=vt[32 * g : 32 * g + 32, 0:J],
                    rhs=w2pad_sb[grp][32 * g : 32 * g + 32, :],
                    tile_position=(32 * g, 0),
                )
                nc.tensor.matmul(
                    out=uvm_ps[0:J, h : h + 1],
                    lhsT=vt[32 * g : 32 * g + 32, 100:200],
                    rhs=w2pad_sb[grp][32 * g : 32 * g + 32, :],
                    tile_position=(32 * g, 0),
                )
            uvj_sb = pp.tile([J, H], F32, tag="uvj")
            nc.vector.tensor_copy(out=uvj_sb, in_=uvj_ps[0:J, 0:H])
            uvm_sb = pp.tile([M, H], F32, tag="uvm")
            nc.vector.tensor_copy(out=uvm_sb, in_=uvm_ps[0:M, 0:H])

            # per head: o_ps = [AT|CT | BT|DT(+zeros)], one exp, uv scales
            eE = []
            for h in range(H):
                grp, g = divmod(h, 4)
                kt, qt = pT_sb[("k", grp)], pT_sb[("q", grp)]
                ps = ps_s1.tile([D, 240], F32, tag="s1")
                nc.tensor.matmul(
                    out=ps[0:J, 0:120],
                    lhsT=kt[32 * g : 32 * g + 32, 0:J],
                    rhs=qt[32 * g : 32 * g + 32, 0:120],
                    tile_position=(32 * g, 0),
                )
                nc.tensor.matmul(
                    out=ps[0:J, 120:240],
                    lhsT=kt[32 * g : 32 * g + 32, 100:200],
                    rhs=qt[32 * g : 32 * g + 32, 0:120],
                    tile_position=(32 * g, 0),
                )
                e1 = rp.tile([D, 280], F32, tag="eE")
                nc.scalar.activation(
                    out=e1[0:J, 0:240], in_=ps[0:J, 0:240], func=AF.Exp, scale=INV_SQ
                )
                nc.vector.tensor_scalar_mul(
                    out=e1[0:J, 240:260],
                    in0=e1[0:J, 100:120],
                    scalar1=uvj_sb[:, h : h + 1],
                )
                nc.vector.tensor_scalar_mul(
                    out=e1[0:M, 260:280],
                    in0=e1[0:M, 220:240],
                    scalar1=uvm_sb[:, h : h + 1],
                )
                eE.append(e1)

            # mm4: [SF|Nm] = eBT.T @ [eDT | eDT*uvm]   (K=20, (32,128))
            f_ps = ps_big.tile([D, 8, 40], F32, tag="big")
            for h in range(H):
                nc.tensor.matmul(
                    out=f_ps[0:J, h, :],
                    lhsT=eE[h][0:M, 120:220],
                    rhs=_chunk2(eE[h][0:M, 220:240], 40),
                )

            def pmh(ap3):  # [p, h, m] -> [p, m, h]
                return ap3.rearrange("p h m -> p m h")

            # F-side combine first: overlaps the mm3 stream on PE
            rF = pp.tile([J, M, H], F32, tag="rF")
            nc.vector.reciprocal(out=rF, in_=pmh(f_ps[0:J, :, 0:M]))
            d2 = pp.tile([J, M, H], F32, tag="d2")
            nc.vector.tensor_mul(out=d2, in0=pmh(f_ps[0:J, :, M : 2 * M]), in1=rF)

            # mm3: [SE|Nj] = eAT.T @ [eCT | eCT*uvj]   (K=100, (128,128))
            s_ps = ps_big.tile([D, 8, 40], F32, tag="big")
            for h in range(H):
                nc.tensor.matmul(
                    out=s_ps[0:J, h, :],
                    lhsT=eE[h][0:J, 0:J],
                    rhs=_chunk2(eE[h][0:J, 100:120], 140),
                )

            # ---- combine: sum_h Nj/SE + Nm/SF ---------------------------
            rE = pp.tile([J, M, H], F32, tag="rE")
            nc.vector.reciprocal(out=rE, in_=pmh(s_ps[0:J, :, 0:M]))
            c8 = pp.tile([J, M, H], F32, tag="c8")
            nc.vector.scalar_tensor_tensor(
                out=c8, in0=pmh(s_ps[0:J, :, M : 2 * M]), scalar=1.0, in1=rE,
                op0=OP.mult, op1=OP.mult,
            )
            nc.vector.tensor_add(out=c8, in0=c8, in1=d2)
            c1 = pp.tile([J, M], F32, tag="c1")
            nc.vector.reduce_sum(out=c1, in_=c8, axis=AX.X)

            # tanh chain via exp (no ACT table switch):
            # logits ~ mask - 20/(exp(2*(c1+bias)/sqrt(D)) + 1)  (+const)
            u = pp.tile([J, M], F32, tag="u")
            nc.scalar.activation(out=u, in_=c1, func=AF.Exp, scale=2.0 / SD, bias=biasb)
            t1 = pp.tile([J, M], F32, tag="t1")
            nc.scalar.add(out=t1, in_=u, add=1.0)
            r = pp.tile([J, M], F32, tag="r")
            nc.vector.reciprocal(out=r, in_=t1)
            arg = pp.tile([J, M], F32, tag="arg")
            nc.vector.scalar_tensor_tensor(
                out=arg, in0=r, scalar=-20.0, in1=mask_v, op0=OP.mult, op1=OP.add
            )
            e_sb = pp.tile([J, M], F32, tag="e")
            s_row = pp.tile([J, 1], F32, tag="srow")
            nc.scalar.activation(
                out=e_sb, in_=arg, func=AF.Exp, scale=1.0, accum_out=s_row
            )
            totb_ps = ps_big.tile([D, 320], F32, tag="big")
            nc.tensor.matmul(out=totb_ps[0:J, 0:1], lhsT=ones_sb[0:J, 0:J], rhs=s_row)
            rtot = pp.tile([J, 1], F32, tag="rtot")
            nc.vector.reciprocal(out=rtot, in_=totb_ps[0:J, 0:1])
            out_t = pp.tile([J, M], F32, tag="outt")
            nc.vector.tensor_scalar_mul(out=out_t, in0=e_sb, scalar1=rtot)
            nc.sync.dma_start(out=out_d[:], in_=out_t)

    _split_multi_waits(nc)
    return nc


_NC = None
last_results = None


def kernel(**inputs):
    global _NC, last_results
    _install_drain_patch()
    if _NC is None:
        _NC = _build()

    wqkv = np.empty((D, 7, D), np.float32)
    for i, nm in enumerate(("Wq3", "Wk", "Wv")):
        w = np.asarray(inputs[nm], np.float32)
        wqkv[:, 2 * i, :] = w[:D]
        wqkv[:, 2 * i + 1, :] = w[D:]
    wqkv[:, 6, :] = np.asarray(inputs["Wmhc"], np.float32)

    ed_base = np.zeros((D, EDATA_W), np.float32)
    ed_base[:, SW0] = np.asarray(inputs["b_mhc"], np.float32).reshape(D)
    ed_base[:, SW0 + 1] = np.asarray(inputs["Wshc"], np.float32).reshape(D)
    ed_base[0, SW0 + 2] = np.float32(np.asarray(inputs["b_shc"]).reshape(-1)[0])

    ejs = np.asarray(inputs["encoded_job"], np.float32)
    ems = np.asarray(inputs["encoded_machine"], np.float32)
    msks = np.asarray(inputs["ninf_mask"], np.float32)

    in_maps = []
    for b in range(B):
        ed = ed_base.copy()
        ed[0:J, EJ0 : EJ0 + D] = ejs[b]
        ed[0:M, EM0 : EM0 + D] = ems[b]
        ed[0:J, MK0 : MK0 + M] = msks[b]
        in_maps.append({"wqkv": wqkv, "edata": ed})

    last_results = run_bass_kernel_spmd(_NC, in_maps, core_ids=list(range(B)))
    out = np.stack(
        [last_results.results[b]["out"].reshape(J * M) for b in range(B)]
    )
    return out.astype(np.float32)
